# revision 1
# baseline (speedup 1.0000x reference)
"""Trainium2 Bass kernel for CSNetModel GNN message passing (8 NeuronCores).

Strategy: shard destination nodes across the 8 cores (12500 each). Each layer's
segment_sum is computed with one-hot matmuls on the tensor engine over
host-sorted edge chunks; per-edge features are fetched with indirect DMA
gathers from replicated (AllGather'd) bf16 node tables. Feature transforms are
fused before aggregation (GCN: gather pre-transformed tables) or after
(RGCN/Hetero: per-relation PSUM banks + weight matmuls). All index arithmetic
is done on the host; the device program is identical across cores (SPMD), with
per-core edge data padded to a uniform chunk/segment schedule.
"""
import math
import numpy as np
import ml_dtypes

import jax
from jax.sharding import Mesh, PartitionSpec, NamedSharding
from jax.experimental.shard_map import shard_map

import concourse.bass as bass
import concourse.bacc as bacc
import concourse.tile as tile
import concourse.mybir as mybir
from concourse.bass2jax import (_bass_exec_p, fast_dispatch_compile,
                                install_neuronx_cc_hook, partition_id_tensor)

F32 = mybir.dt.float32
BF16 = mybir.dt.bfloat16
I32 = mybir.dt.int32

NCORES = 8
N = 100000
NLOC = N // NCORES          # 12500
D = 128
NTILE = (NLOC + 127) // 128  # 98
LAST_W = NLOC - (NTILE - 1) * 128  # 84
R_HET = 4
R_RG = 8

TDT = BF16                   # table / matmul dtype
TNP = ml_dtypes.bfloat16

ALIGN = {"gcn1": False, "gcn2": False, "rg1": False, "rg2": False,
         "het1": False, "het2": False}


def _tw(t):
    return 128 if t < NTILE - 1 else LAST_W


# ---------------------------------------------------------------------------
# Host-side edge packing
# ---------------------------------------------------------------------------

def pack_layer(src, dst, rel, R, align):
    """Build SPMD-uniform chunk/segment schedule for one layer-graph.

    src, dst: int arrays [E] (global node ids); rel: int array [E] or None.
    Returns dict with nchunk, nseg, groups (ordered list), and per-core
    idx_mat [128, nchunk] int32 / dl_mat [128, nseg] float32.
    """
    src = np.asarray(src).astype(np.int64)
    dst = np.asarray(dst).astype(np.int64)
    rel = np.zeros_like(src) if rel is None else np.asarray(rel).astype(np.int64)
    core = dst // NLOC
    dl = dst % NLOC
    tl = dl // 128
    dloc = dl % 128
    g = tl * R + rel
    NG = NTILE * R

    counts = np.zeros((NCORES, NG), np.int64)
    percore = []
    for c in range(NCORES):
        m = core == c
        gc = g[m]
        order = np.argsort(gc, kind="stable")
        gc = gc[order]
        percore.append((gc, src[m][order], dloc[m][order]))
        counts[c] = np.bincount(gc, minlength=NG)
    NE = counts.max(axis=0)

    if align:
        sizes = ((NE + 127) // 128) * 128
    else:
        sizes = NE.copy()
    off = np.zeros(NG + 1, np.int64)
    np.cumsum(sizes, out=off[1:])
    total = int(off[-1])
    nchunk = (total + 127) // 128
    tot_pad = nchunk * 128

    groups = []
    nseg = 0
    for gi in range(NG):
        ne = int(NE[gi])
        if ne == 0:
            continue
        lo, hi = int(off[gi]), int(off[gi]) + ne
        segs = []
        for k in range(lo // 128, (hi - 1) // 128 + 1):
            segs.append((k, nseg))
            nseg += 1
        groups.append({"t": gi // R, "r": gi % R, "lo": lo, "hi": hi, "segs": segs})

    group_of = {(grp["t"], grp["r"]): grp for grp in groups}

    idx_mats, dl_mats = [], []
    starts = off[:-1]
    for c in range(NCORES):
        gc, srcs, dlocs = percore[c]
        first_occ = np.searchsorted(gc, np.arange(NG))
        pos = starts[gc] + (np.arange(len(gc)) - first_occ[gc])
        idx_flat = np.zeros(tot_pad, np.int32)
        idx_flat[pos] = srcs
        dl_flat = np.full(tot_pad, -1.0, np.float32)
        dl_flat[pos] = dlocs
        idx_mats.append(np.ascontiguousarray(idx_flat.reshape(nchunk, 128).T))
        dl_mat = np.full((128, max(nseg, 1)), -1.0, np.float32)
        for grp in groups:
            for (k, col) in grp["segs"]:
                s = max(grp["lo"], k * 128)
                e = min(grp["hi"], (k + 1) * 128)
                colv = np.full(128, -1.0, np.float32)
                colv[s - k * 128:e - k * 128] = dl_flat[s:e]
                dl_mat[:, col] = colv
        dl_mats.append(dl_mat)

    return {"nchunk": nchunk, "nseg": max(nseg, 1), "groups": groups,
            "group_of": group_of, "idx": idx_mats, "dl": dl_mats}


# ---------------------------------------------------------------------------
# Device program
# ---------------------------------------------------------------------------

def build_program(plans, stages=("prep", "ag01", "l1", "ag234", "l2"),
                  loop_r=None, scopes=False):
    from contextlib import nullcontext
    stages = set(stages)
    noag = "noag" in stages
    nc = bacc.Bacc("TRN2", target_bir_lowering=False, debug=False,
                   num_devices=NCORES)

    # --- external inputs (per core) ---
    ext = {}

    def din(name, shape, dt):
        ext[name] = nc.dram_tensor(name, list(shape), dt, kind="ExternalInput")
        return ext[name]

    emb_sl = din("emb_sl", [NLOC, D], TDT)
    gcn_W1 = din("gcn_W1", [D, D], TDT)
    gcn_W2 = din("gcn_W2", [D, D], TDT)
    gcn_b1 = din("gcn_b1", [D, 1], F32)
    gcn_b2r = din("gcn_b2r", [D, D], F32)          # row-broadcast bias
    rg_W1 = din("rg_W1", [D, R_RG * D], TDT)
    rg_W2 = din("rg_W2", [D, R_RG * D], TDT)
    rg_loop1 = din("rg_loop1", [D, D], TDT)
    rg_loop2 = din("rg_loop2", [D, D], TDT)
    rg_b1 = din("rg_b1", [D, 1], F32)
    rg_b2 = din("rg_b2", [D, 1], F32)
    het_W1 = din("het_W1", [D, R_HET * D], TDT)
    het_W2 = din("het_W2", [D, R_HET * D], TDT)    # pre-scaled by 0.25 on host
    het_b1 = din("het_b1", [D, R_HET], F32)
    het_b2 = din("het_b2", [D, R_HET], F32)
    iota_in = din("iota", [D, D], TDT)
    ident_b = din("ident_b", [D, D], TDT)
    ident_f = din("ident_f", [D, D], F32)
    for lname in ("gcn1", "gcn2", "rg1", "rg2", "het1", "het2"):
        p = plans[lname]
        din(f"idx_{lname}", [128, p["nchunk"]], I32)
        din(f"dl_{lname}", [128, p["nseg"]], TDT)

    hcf_out = nc.dram_tensor("hcf", [NLOC, D], F32, kind="ExternalOutput")
    hc_out = nc.dram_tensor("hc", [NLOC, D], F32, kind="ExternalOutput")
    hs_out = nc.dram_tensor("hs", [NLOC, D], F32, kind="ExternalOutput")

    if noag:
        ext_tabs = {}
        for nm in ("emb_full_in", "t1_full_in", "t2_full_in", "h1_full_in",
                   "hs1_full_in"):
            ext_tabs[nm] = nc.dram_tensor(nm, [N, D], TDT, kind="ExternalInput")

    Tanh = mybir.ActivationFunctionType.Tanh
    AG = "AllGather"
    RGROUPS = [list(range(NCORES))]

    with tile.TileContext(nc) as tc:
        with tc.tile_pool(name="consts", bufs=1) as cp, \
             tc.tile_pool(name="gat", bufs=8) as gp, \
             tc.tile_pool(name="oh", bufs=12) as ohp, \
             tc.tile_pool(name="work", bufs=6) as wp, \
             tc.tile_pool(name="psb", bufs=3, space="PSUM") as psb, \
             tc.tile_pool(name="pss", bufs=3, space="PSUM") as pss, \
             tc.tile_pool(name="ptr", bufs=2, space="PSUM") as ptr, \
             tc.tile_pool(name="dram", bufs=1, space="DRAM") as dp:

            # --- constant tiles ---
            def load_const(name, shape, dt):
                t = cp.tile(list(shape), dt, tag=name)
                nc.sync.dma_start(out=t[:], in_=ext[name][:])
                return t

            iota_t = load_const("iota", [D, D], TDT)
            identb_t = load_const("ident_b", [D, D], TDT)
            identf_t = load_const("ident_f", [D, D], F32)
            gW1_t = load_const("gcn_W1", [D, D], TDT)
            gW2_t = load_const("gcn_W2", [D, D], TDT)
            gb1_t = load_const("gcn_b1", [D, 1], F32)
            gb2r_t = load_const("gcn_b2r", [D, D], F32)
            rW1_t = load_const("rg_W1", [D, R_RG * D], TDT)
            rW2_t = load_const("rg_W2", [D, R_RG * D], TDT)
            rL1_t = load_const("rg_loop1", [D, D], TDT)
            rL2_t = load_const("rg_loop2", [D, D], TDT)
            rb1_t = load_const("rg_b1", [D, 1], F32)
            rb2_t = load_const("rg_b2", [D, 1], F32)
            hW1_t = load_const("het_W1", [D, R_HET * D], TDT)
            hW2_t = load_const("het_W2", [D, R_HET * D], TDT)
            hb1_t = load_const("het_b1", [D, R_HET], F32)
            hb2_t = load_const("het_b2", [D, R_HET], F32)
            meta = {}
            for lname in ("gcn1", "gcn2", "rg1", "rg2", "het1", "het2"):
                p = plans[lname]
                meta[lname] = (
                    load_const(f"idx_{lname}", [128, p["nchunk"]], I32),
                    load_const(f"dl_{lname}", [128, p["nseg"]], TDT),
                )

            # --- internal DRAM ---
            emb_bounce = dp.tile([NLOC, D], TDT, tag="emb_b")
            emb_full = dp.tile([N, D], TDT, tag="emb_f", addr_space="Shared")
            loop_ctx = (tc.For_i(0, loop_r, 1)
                        if (loop_r and "agrep" not in stages) else None)
            if loop_ctx:
                loop_ctx.__enter__()
            t1_bounce = dp.tile([NLOC, D], TDT, tag="t1_b")
            t1_full = dp.tile([N, D], TDT, tag="t1_f", addr_space="Shared")
            t2_bounce = dp.tile([NLOC, D], TDT, tag="t2_b")
            t2_full = dp.tile([N, D], TDT, tag="t2_f", addr_space="Shared")
            h1_bounce = dp.tile([NLOC, D], TDT, tag="h1_b")
            h1_full = dp.tile([N, D], TDT, tag="h1_f", addr_space="Shared")
            hs1_bounce = dp.tile([NLOC, D], TDT, tag="hs1_b")
            hs1_full = dp.tile([N, D], TDT, tag="hs1_f", addr_space="Shared")
            embT_dram = dp.tile([D, NLOC], TDT, tag="embT")
            h1T_dram = dp.tile([D, NLOC], TDT, tag="h1T")

            def _scope(name):
                return nc.named_scope(name, notify=True) if scopes \
                    else nullcontext()

            def _ag(src, dst):
                nc.gpsimd.collective_compute(
                    AG, mybir.AluOpType.bypass, replica_groups=RGROUPS,
                    ins=[src.opt()], outs=[dst.opt()])

            # --- prep: embT tiles, T1 = emb @ W1, bounces ---
            prep_scope = _scope("prep")
            prep_scope.__enter__()
            nc.sync.dma_start(out=emb_bounce[:], in_=emb_sl[:])
            # emb AG fires as soon as the bounce copy lands, overlapping the
            # prep matmul loop; rg1/het1 gathers can then start during prep.
            if "ag01" in stages:
                _ag(emb_bounce, emb_full)
            for t in range(NTILE):
                w = _tw(t)
                e_sb = wp.tile([128, D], TDT, tag="embt")
                if w < 128:
                    nc.vector.memset(e_sb[:], 0.0)
                nc.sync.dma_start(out=e_sb[:w, :], in_=emb_sl[t * 128:t * 128 + w, :])
                trp = ptr.tile([128, D], TDT, tag="ptr")
                nc.tensor.transpose(out=trp[:], in_=e_sb[:], identity=identb_t[:])
                eT = wp.tile([128, D], TDT, tag="eT")
                nc.vector.tensor_copy(out=eT[:], in_=trp[:])
                nc.sync.dma_start(out=embT_dram[:, t * 128:t * 128 + w],
                                  in_=eT[:, :w])
                t1p = pss.tile([128, D], F32, tag="pss")
                nc.tensor.matmul(out=t1p[:], lhsT=eT[:], rhs=gW1_t[:],
                                 start=True, stop=True)
                t1sb = wp.tile([128, D], TDT, tag="t1sb")
                nc.vector.tensor_copy(out=t1sb[:], in_=t1p[:])
                nc.sync.dma_start(out=t1_bounce[t * 128:t * 128 + w, :],
                                  in_=t1sb[:w, :])

            prep_scope.__exit__(None, None, None)

            if "ag01" in stages:
                with _scope("ag01"):
                    _ag(t1_bounce, t1_full)

            # --- shared layer machinery ---
            # GM chunks per indirect gather: the SWDGE fixed overhead
            # (~1us/instruction on the Pool engine) dominates per-chunk
            # gathers, so batch GM row-blocks into one descriptor-gen pass.
            GM = 1

            def gather_fn(lname, table):
                idx_t, _ = meta[lname]
                nchunk = plans[lname]["nchunk"]
                cache = {}

                def gather(k):
                    s = k // GM
                    if s not in cache:
                        lo = s * GM
                        w = min(GM, nchunk - lo)
                        gt = gp.tile([128, GM * D], TDT, tag="gat")
                        nc.gpsimd.indirect_dma_start(
                            out=gt[:, :w * D], out_offset=None, in_=table[:],
                            in_offset=bass.IndirectOffsetOnAxis(
                                ap=idx_t[:, lo:lo + w], axis=0))
                        cache[s] = gt
                    return cache[s][:, (k % GM) * D:(k % GM + 1) * D]
                return gather

            def onehot(lname, col):
                _, dl_t = meta[lname]
                oh = ohp.tile([128, D], TDT, tag="oh")
                nc.vector.tensor_tensor(
                    out=oh[:], in0=dl_t[:, col:col + 1].to_broadcast([128, D]),
                    in1=iota_t[:], op=mybir.AluOpType.is_equal)
                return oh

            def accum_group(lname, gather, grp, bank, bcol, transposed):
                """Accumulate one (tile, rel) group into bank[:, bcol:bcol+128].

                transposed=True -> out[f, d] (lhsT=msgs, rhs=onehot)
                transposed=False -> out[d, f] (lhsT=onehot, rhs=msgs)
                """
                segs = grp["segs"]
                for si, (k, col) in enumerate(segs):
                    gt = gather(k)          # AP slice [128, D]
                    oh = onehot(lname, col)
                    lhsT, rhs = (gt, oh[:]) if transposed else (oh[:], gt)
                    nc.tensor.matmul(out=bank[:, bcol:bcol + 128],
                                     lhsT=lhsT, rhs=rhs,
                                     start=(si == 0), stop=(si == len(segs) - 1))

            # =========== GCN layer 1 (gathers T1; aggT supertiles) ==========
            def emit_gcn1():
                lname = "gcn1"
                plan = plans[lname]
                gather = gather_fn(lname, t1_full)
                for st in range((NTILE + 3) // 4):
                    tls = list(range(st * 4, min(st * 4 + 4, NTILE)))
                    bank = psb.tile([128, 512], F32, tag="psb")
                    for j, t in enumerate(tls):
                        grp = plan["group_of"].get((t, 0))
                        if grp is None:
                            nc.vector.memset(bank[:, j * 128:(j + 1) * 128], 0.0)
                            continue
                        accum_group(lname, gather, grp, bank, j * 128, True)
                    w = 128 * len(tls)
                    h1T = wp.tile([128, 512], TDT, tag="h1Tst")
                    nc.scalar.activation(h1T[:, :w], bank[:, :w], Tanh,
                                         bias=gb1_t[:], scale=1.0)
                    for j, t in enumerate(tls):
                        tp = pss.tile([128, D], F32, tag="pss")
                        nc.tensor.matmul(out=tp[:],
                                         lhsT=h1T[:, j * 128:(j + 1) * 128],
                                         rhs=gW2_t[:], start=True, stop=True)
                        tsb = wp.tile([128, D], TDT, tag="t2sb")
                        nc.vector.tensor_copy(out=tsb[:], in_=tp[:])
                        nc.sync.dma_start(
                            out=t2_bounce[t * 128:t * 128 + _tw(t), :],
                            in_=tsb[:_tw(t), :])

            # =========== GCN layer 2 (gathers T2; agg per tile) =============
            def emit_gcn2():
                lname = "gcn2"
                plan = plans[lname]
                gather = gather_fn(lname, t2_full)
                for t in range(NTILE):
                    grp = plan["group_of"].get((t, 0))
                    pt = pss.tile([128, D], F32, tag="pss")
                    if grp is None:
                        nc.vector.memset(pt[:], 0.0)
                    else:
                        accum_group(lname, gather, grp, pt, 0, False)
                    tmp = wp.tile([128, D], F32, tag="g2tmp")
                    nc.vector.tensor_add(out=tmp[:], in0=pt[:], in1=gb2r_t[:])
                    ot = wp.tile([128, D], F32, tag="g2out")
                    nc.scalar.activation(ot[:], tmp[:], Tanh)
                    nc.sync.dma_start(out=hcf_out[t * 128:t * 128 + _tw(t), :],
                                      in_=ot[:_tw(t), :])

            # =========== RGCN layer (B banks per rel + transforms) ==========
            def emit_rg(lname, table, xT_src, W_t, loop_t, b_t, first):
                plan = plans[lname]
                gather = gather_fn(lname, table)
                for t in range(NTILE):
                    w = _tw(t)
                    quads = []
                    for qi in range(2):
                        q = psb.tile([128, 512], F32, tag="psb")
                        quads.append(q)
                    for r in range(R_RG):
                        grp = plan["group_of"].get((t, r))
                        q, qc = quads[r // 4], (r % 4) * 128
                        if grp is None:
                            nc.vector.memset(q[:, qc:qc + 128], 0.0)
                        else:
                            accum_group(lname, gather, grp, q, qc, True)
                    stages = []
                    for qi in range(2):
                        s = wp.tile([128, 512], TDT, tag="stage")
                        nc.vector.tensor_copy(out=s[:], in_=quads[qi][:])
                        stages.append(s)
                    xT_t = wp.tile([128, D], TDT, tag="xTt")
                    nc.sync.dma_start(out=xT_t[:, :w],
                                      in_=xT_src[:, t * 128:t * 128 + w])
                    ot = pss.tile([128, D], F32, tag="pss")
                    nc.tensor.matmul(out=ot[:], lhsT=loop_t[:], rhs=xT_t[:],
                                     start=True, stop=False)
                    for r in range(R_RG):
                        nc.tensor.matmul(
                            out=ot[:], lhsT=W_t[:, r * 128:(r + 1) * 128],
                            rhs=stages[r // 4][:, (r % 4) * 128:(r % 4 + 1) * 128],
                            start=False, stop=(r == R_RG - 1))
                    if first:
                        hT = wp.tile([128, D], TDT, tag="hTb")
                        nc.scalar.activation(hT[:], ot[:], Tanh, bias=b_t[:],
                                             scale=1.0)
                        nc.sync.dma_start(
                            out=h1T_dram[:, t * 128:t * 128 + w], in_=hT[:, :w])
                        trp = ptr.tile([128, D], TDT, tag="ptr")
                        nc.tensor.transpose(out=trp[:], in_=hT[:],
                                            identity=identb_t[:])
                        hsb = wp.tile([128, D], TDT, tag="hsbb")
                        nc.vector.tensor_copy(out=hsb[:], in_=trp[:])
                        nc.sync.dma_start(out=h1_bounce[t * 128:t * 128 + w, :],
                                          in_=hsb[:w, :])
                    else:
                        hTf = wp.tile([128, D], F32, tag="hTf")
                        nc.scalar.activation(hTf[:], ot[:], Tanh, bias=b_t[:],
                                             scale=1.0)
                        trp = ptr.tile([128, D], F32, tag="ptr")
                        nc.tensor.transpose(out=trp[:], in_=hTf[:],
                                            identity=identf_t[:])
                        hsb = wp.tile([128, D], F32, tag="hsbf")
                        nc.vector.tensor_copy(out=hsb[:], in_=trp[:])
                        nc.sync.dma_start(out=hc_out[t * 128:t * 128 + w, :],
                                          in_=hsb[:w, :])

            # =========== Hetero layer (4 rels, mean of tanh) ================
            def emit_het(lname, table, W_t, b_t, first):
                plan = plans[lname]
                gather = gather_fn(lname, table)
                for t in range(NTILE):
                    w = _tw(t)
                    quad = psb.tile([128, 512], F32, tag="psb")
                    for r in range(R_HET):
                        grp = plan["group_of"].get((t, r))
                        if grp is None:
                            nc.vector.memset(quad[:, r * 128:(r + 1) * 128], 0.0)
                        else:
                            accum_group(lname, gather, grp, quad, r * 128, True)
                    stage = wp.tile([128, 512], TDT, tag="stage")
                    nc.vector.tensor_copy(out=stage[:], in_=quad[:])
                    acc = wp.tile([128, D], F32, tag="hacc")
                    for r in range(R_HET):
                        otr = pss.tile([128, D], F32, tag="pss")
                        nc.tensor.matmul(
                            out=otr[:], lhsT=W_t[:, r * 128:(r + 1) * 128],
                            rhs=stage[:, r * 128:(r + 1) * 128],
                            start=True, stop=True)
                        if r == 0:
                            nc.scalar.activation(acc[:], otr[:], Tanh,
                                                 bias=b_t[:, 0:1], scale=1.0)
                        else:
                            tmp = wp.tile([128, D], F32, tag="htmp")
                            nc.scalar.activation(tmp[:], otr[:], Tanh,
                                                 bias=b_t[:, r:r + 1], scale=1.0)
                            nc.vector.tensor_add(out=acc[:], in0=acc[:],
                                                 in1=tmp[:])
                    if first:
                        # no 0.25 scale: folded into het_W2 on host
                        hsT = wp.tile([128, D], TDT, tag="hTb")
                        nc.vector.tensor_copy(out=hsT[:], in_=acc[:])
                        trp = ptr.tile([128, D], TDT, tag="ptr")
                        nc.tensor.transpose(out=trp[:], in_=hsT[:],
                                            identity=identb_t[:])
                        hsb = wp.tile([128, D], TDT, tag="hsbb")
                        nc.vector.tensor_copy(out=hsb[:], in_=trp[:])
                        nc.sync.dma_start(out=hs1_bounce[t * 128:t * 128 + w, :],
                                          in_=hsb[:w, :])
                    else:
                        hsT = wp.tile([128, D], F32, tag="hTf")
                        nc.vector.tensor_scalar_mul(hsT[:], acc[:], 0.25)
                        trp = ptr.tile([128, D], F32, tag="ptr")
                        nc.tensor.transpose(out=trp[:], in_=hsT[:],
                                            identity=identf_t[:])
                        hsb = wp.tile([128, D], F32, tag="hsbf")
                        nc.vector.tensor_copy(out=hsb[:], in_=trp[:])
                        nc.sync.dma_start(out=hs_out[t * 128:t * 128 + w, :],
                                          in_=hsb[:w, :])

            # --- emit layers ---
            if noag:
                emb_full = ext_tabs["emb_full_in"]
                t1_full = ext_tabs["t1_full_in"]
                t2_full = ext_tabs["t2_full_in"]
                h1_full = ext_tabs["h1_full_in"]
                hs1_full = ext_tabs["hs1_full_in"]
            # Each layer-1 AG fires right after its producer layer so the
            # transfer overlaps the remaining layer-1 compute.
            if "l1" in stages or "l1rg" in stages:
                with _scope("l1_rg"):
                    emit_rg("rg1", emb_full, embT_dram, rW1_t, rL1_t, rb1_t,
                            True)
                if "ag234" in stages:
                    _ag(h1_bounce, h1_full)
            if "l1" in stages or "l1het" in stages:
                with _scope("l1_het"):
                    emit_het("het1", emb_full, hW1_t, hb1_t, True)
                if "ag234" in stages:
                    _ag(hs1_bounce, hs1_full)
            if "l1" in stages or "l1gcn" in stages:
                with _scope("l1_gcn"):
                    emit_gcn1()
                if "ag234" in stages:
                    _ag(t2_bounce, t2_full)

            if "l2" in stages or "l2rg" in stages:
                with _scope("l2_rg"):
                    emit_rg("rg2", h1_full, h1T_dram, rW2_t, rL2_t, rb2_t,
                            False)
            if "l2" in stages or "l2het" in stages:
                with _scope("l2_het"):
                    emit_het("het2", hs1_full, hW2_t, hb2_t, False)
            if "l2" in stages or "l2gcn" in stages:
                with _scope("l2_gcn"):
                    emit_gcn2()
            if loop_ctx:
                loop_ctx.__exit__(None, None, None)

    nc.compile()
    return nc


# ---------------------------------------------------------------------------
# Runner (PJRT via axon)
# ---------------------------------------------------------------------------

class _Runner:
    """One execute + one await per run.

    The axon tunnel costs ~70ms per client-side await RPC (independent of
    data size or device work), so the run path is: a single bass_exec
    dispatch over all 8 cores, then a single jax.block_until_ready. The
    kernel writes every element of each ExternalOutput, so no pre-zeroed
    output operands are passed (PJRT allocates the result buffers and the
    NEFF fills them).
    """

    def __init__(self, nc, n_cores):
        install_neuronx_cc_hook()
        self.n_cores = n_cores
        partition_name = (nc.partition_id_tensor.name
                          if nc.partition_id_tensor else None)
        in_names, out_names, out_avals = [], [], []
        for alloc in nc.m.functions[0].allocations:
            if not isinstance(alloc, mybir.MemoryLocationSet):
                continue
            name = alloc.memorylocations[0].name
            if alloc.kind == "ExternalInput":
                if name != partition_name:
                    in_names.append(name)
            elif alloc.kind == "ExternalOutput":
                shape = tuple(alloc.tensor_shape)
                dtype = mybir.dt.np(alloc.dtype)
                out_avals.append(jax.core.ShapedArray(shape, dtype))
                out_names.append(name)
        self.in_names, self.out_names = in_names, out_names
        self.out_avals = out_avals
        n_params, n_outs = len(in_names), len(out_avals)
        all_in = list(in_names)
        if partition_name is not None:
            all_in.append(partition_name)

        def _body(*args):
            operands = list(args)
            if partition_name is not None:
                operands.append(partition_id_tensor())
            return tuple(_bass_exec_p.bind(
                *operands, out_avals=tuple(out_avals), in_names=tuple(all_in),
                out_names=tuple(out_names), lowering_input_output_aliases=(),
                sim_require_finite=True, sim_require_nnan=True, nc=nc))

        devices = jax.devices()[:n_cores]
        self.mesh = Mesh(np.asarray(devices), ("core",))
        in_specs = (PartitionSpec("core"),) * n_params
        out_specs = (PartitionSpec("core"),) * n_outs
        self._body = _body
        self._specs = (in_specs, out_specs)
        self.fn = None
        self.sharding = NamedSharding(self.mesh, PartitionSpec("core"))

    def _ensure_compiled(self):
        # AOT-compile with the bass effect suppressed so repeat dispatches go
        # through JAX's C++ fast path (the effectful path adds per-call
        # Python token machinery).
        if self.fn is None:
            in_specs, out_specs = self._specs

            def _compile():
                return jax.jit(
                    shard_map(self._body, mesh=self.mesh, in_specs=in_specs,
                              out_specs=out_specs, check_rep=False),
                    keep_unused=True).lower(*self.dev_in).compile()

            self.fn = fast_dispatch_compile(_compile)

    def put_inputs(self, in_maps):
        n = self.n_cores
        per_core = [[np.asarray(m[k]) for k in self.in_names] for m in in_maps]
        self.dev_in = [
            jax.device_put(
                np.concatenate([per_core[c][i] for c in range(n)], axis=0),
                self.sharding)
            for i in range(len(self.in_names))
        ]
        jax.block_until_ready(self.dev_in)

    def run(self, fetch=True):
        n = self.n_cores
        self._ensure_compiled()
        outs = self.fn(*self.dev_in)
        jax.block_until_ready(outs)
        if not fetch:
            return None
        return [
            {name: np.asarray(outs[i]).reshape(n, *self.out_avals[i].shape)[c]
             for i, name in enumerate(self.out_names)}
            for c in range(n)
        ]


# ---------------------------------------------------------------------------
# Entry point
# ---------------------------------------------------------------------------

_LAST_RUNNER = None


def build_all(inputs, stages=("prep", "ag01", "l1", "ag234", "l2"),
              loop_r=None, scopes=False):
    """Pack edges + build program + per-core input maps. Returns (nc, in_maps)."""
    gcn_src1 = inputs["gcn_src1"]; gcn_dst1 = inputs["gcn_dst1"]
    gcn_src2 = inputs["gcn_src2"]; gcn_dst2 = inputs["gcn_dst2"]
    rg_src1 = inputs["rg_src1"]; rg_dst1 = inputs["rg_dst1"]
    rg_et1 = inputs["rg_et1"]
    rg_src2 = inputs["rg_src2"]; rg_dst2 = inputs["rg_dst2"]
    rg_et2 = inputs["rg_et2"]
    het_src1 = inputs["het_src1"]; het_dst1 = inputs["het_dst1"]
    het_src2 = inputs["het_src2"]; het_dst2 = inputs["het_dst2"]
    emb = inputs["emb"]
    gcn_W1 = inputs["gcn_W1"]; gcn_b1 = inputs["gcn_b1"]
    gcn_W2 = inputs["gcn_W2"]; gcn_b2 = inputs["gcn_b2"]
    rg_W1 = inputs["rg_W1"]; rg_loop1 = inputs["rg_loop1"]
    rg_b1 = inputs["rg_b1"]
    rg_W2 = inputs["rg_W2"]; rg_loop2 = inputs["rg_loop2"]
    rg_b2 = inputs["rg_b2"]
    het_W1 = inputs["het_W1"]; het_b1 = inputs["het_b1"]
    het_W2 = inputs["het_W2"]; het_b2 = inputs["het_b2"]
    emb = np.asarray(emb, np.float32)

    # hetero edge lists: concatenate the 4 relations with rel tags
    def het_edges(srcs, dsts):
        s = np.concatenate([np.asarray(srcs[r]).ravel() for r in range(R_HET)])
        d = np.concatenate([np.asarray(dsts[r]).ravel() for r in range(R_HET)])
        r = np.concatenate([np.full(np.asarray(srcs[r]).size, r, np.int64)
                            for r in range(R_HET)])
        return s, d, r

    hs1_, hd1_, hr1_ = het_edges(het_src1, het_dst1)
    hs2_, hd2_, hr2_ = het_edges(het_src2, het_dst2)

    plans = {
        "gcn1": pack_layer(gcn_src1, gcn_dst1, None, 1, ALIGN["gcn1"]),
        "gcn2": pack_layer(gcn_src2, gcn_dst2, None, 1, ALIGN["gcn2"]),
        "rg1": pack_layer(rg_src1, rg_dst1, rg_et1, R_RG, ALIGN["rg1"]),
        "rg2": pack_layer(rg_src2, rg_dst2, rg_et2, R_RG, ALIGN["rg2"]),
        "het1": pack_layer(hs1_, hd1_, hr1_, R_HET, ALIGN["het1"]),
        "het2": pack_layer(hs2_, hd2_, hr2_, R_HET, ALIGN["het2"]),
    }

    nc = build_program(plans, stages=stages, loop_r=loop_r, scopes=scopes)

    iota_np = np.broadcast_to(np.arange(D, dtype=np.float32), (D, D))
    shared = {
        "gcn_W1": np.asarray(gcn_W1).astype(TNP),
        "gcn_W2": np.asarray(gcn_W2).astype(TNP),
        "gcn_b1": np.asarray(gcn_b1, np.float32).reshape(D, 1),
        "gcn_b2r": np.broadcast_to(np.asarray(gcn_b2, np.float32), (D, D)).copy(),
        "rg_W1": np.concatenate([np.asarray(rg_W1)[r] for r in range(R_RG)],
                                axis=1).astype(TNP),
        "rg_W2": np.concatenate([np.asarray(rg_W2)[r] for r in range(R_RG)],
                                axis=1).astype(TNP),
        "rg_loop1": np.asarray(rg_loop1).astype(TNP),
        "rg_loop2": np.asarray(rg_loop2).astype(TNP),
        "rg_b1": np.asarray(rg_b1, np.float32).reshape(D, 1),
        "rg_b2": np.asarray(rg_b2, np.float32).reshape(D, 1),
        "het_W1": np.concatenate([np.asarray(het_W1)[r] for r in range(R_HET)],
                                 axis=1).astype(TNP),
        "het_W2": np.concatenate([0.25 * np.asarray(het_W2)[r]
                                  for r in range(R_HET)], axis=1).astype(TNP),
        "het_b1": np.ascontiguousarray(np.asarray(het_b1, np.float32).T),
        "het_b2": np.ascontiguousarray(np.asarray(het_b2, np.float32).T),
        "iota": iota_np.astype(TNP),
        "ident_b": np.eye(D, dtype=TNP),
        "ident_f": np.eye(D, dtype=np.float32),
    }

    in_maps = []
    for c in range(NCORES):
        m = dict(shared)
        m["emb_sl"] = emb[c * NLOC:(c + 1) * NLOC, :].astype(TNP)
        for lname in ("gcn1", "gcn2", "rg1", "rg2", "het1", "het2"):
            m[f"idx_{lname}"] = plans[lname]["idx"][c]
            m[f"dl_{lname}"] = plans[lname]["dl"][c].astype(TNP)
        in_maps.append(m)
    return nc, in_maps


def kernel(gcn_src1, gcn_dst1, gcn_src2, gcn_dst2,
           rg_src1, rg_dst1, rg_et1, rg_src2, rg_dst2, rg_et2,
           het_src1, het_dst1, het_src2, het_dst2,
           emb, gcn_W1, gcn_b1, gcn_W2, gcn_b2,
           rg_W1, rg_loop1, rg_b1, rg_W2, rg_loop2, rg_b2,
           het_W1, het_b1, het_W2, het_b2):
    nc, in_maps = build_all(dict(
        gcn_src1=gcn_src1, gcn_dst1=gcn_dst1, gcn_src2=gcn_src2,
        gcn_dst2=gcn_dst2, rg_src1=rg_src1, rg_dst1=rg_dst1, rg_et1=rg_et1,
        rg_src2=rg_src2, rg_dst2=rg_dst2, rg_et2=rg_et2, het_src1=het_src1,
        het_dst1=het_dst1, het_src2=het_src2, het_dst2=het_dst2, emb=emb,
        gcn_W1=gcn_W1, gcn_b1=gcn_b1, gcn_W2=gcn_W2, gcn_b2=gcn_b2,
        rg_W1=rg_W1, rg_loop1=rg_loop1, rg_b1=rg_b1, rg_W2=rg_W2,
        rg_loop2=rg_loop2, rg_b2=rg_b2, het_W1=het_W1, het_b1=het_b1,
        het_W2=het_W2, het_b2=het_b2))
    runner = _Runner(nc, NCORES)
    global _LAST_RUNNER
    _LAST_RUNNER = runner
    runner.put_inputs(in_maps)
    res = runner.run()

    hcf = np.concatenate([res[c]["hcf"] for c in range(NCORES)], axis=0)
    hc = np.concatenate([res[c]["hc"] for c in range(NCORES)], axis=0)
    hs = np.concatenate([res[c]["hs"] for c in range(NCORES)], axis=0)
    return (hcf, hc, hs)



# revision 42
# speedup vs baseline: 15.0213x; 15.0213x over previous
"""Trainium2 Bass kernel for CSNetModel GNN message passing (8 NeuronCores).

Strategy: shard destination nodes across the 8 cores (12500 each). Each layer's
segment_sum is computed with one-hot matmuls on the tensor engine over
host-sorted edge chunks; per-edge features are fetched with indirect DMA
gathers from replicated (AllGather'd) bf16 node tables. Feature transforms are
fused before aggregation (GCN: gather pre-transformed tables) or after
(RGCN/Hetero: per-relation PSUM banks + weight matmuls). All index arithmetic
is done on the host; the device program is identical across cores (SPMD), with
per-core edge data padded to a uniform chunk/segment schedule.
"""
import math
import numpy as np
import ml_dtypes

import jax
from jax.sharding import Mesh, PartitionSpec, NamedSharding
from jax.experimental.shard_map import shard_map

import concourse.bass as bass
import concourse.bacc as bacc
import concourse.tile as tile
import concourse.mybir as mybir
from concourse.bass2jax import (_bass_exec_p, fast_dispatch_compile,
                                install_neuronx_cc_hook, partition_id_tensor)

F32 = mybir.dt.float32
BF16 = mybir.dt.bfloat16
I32 = mybir.dt.int32
I16 = mybir.dt.int16

NCORES = 8
N = 100000
NLOC = N // NCORES          # 12500
D = 128
NTILE = (NLOC + 127) // 128  # 98
LAST_W = NLOC - (NTILE - 1) * 128  # 84
R_HET = 4
R_RG = 8

TDT = BF16                   # table / matmul dtype
TNP = ml_dtypes.bfloat16

NRHO = 4                     # src-range splits (dma_gather int16 index reach)
RBASE = N // NRHO            # 25000 rows per range (< 32768)
GB = 4                       # tiles per gather batch (one dma_gather per range)


def _tw(t):
    return 128 if t < NTILE - 1 else LAST_W


# ---------------------------------------------------------------------------
# Host-side edge packing
# ---------------------------------------------------------------------------

def pack_layer(src, dst, rel, R):
    """Build SPMD-uniform dma_gather run + segment schedule for one layer.

    Slot stream order: (gather-batch gb of GB tiles, src-range rho, tile,
    rel). Each (gb, rho) run is one dma_gather instruction (int16 indices
    relative to rho*RBASE), padded to a 128-slot chunk grid. Cells
    (rho, t, rel) are padded to the max count across the 8 cores so the
    instruction/matmul schedule is SPMD-uniform; per-core edge content
    lives in the idx/dl data.
    """
    src = np.asarray(src).astype(np.int64)
    dst = np.asarray(dst).astype(np.int64)
    rel = np.zeros_like(src) if rel is None else np.asarray(rel).astype(np.int64)
    core = dst // NLOC
    dl = dst % NLOC
    tl = dl // 128
    dloc = dl % 128
    rho = src // RBASE
    NGB = (NTILE + GB - 1) // GB
    gbv = tl // GB
    cell = ((gbv * NRHO + rho) * NTILE + tl) * R + rel
    NCELL = NGB * NRHO * NTILE * R

    counts = np.zeros((NCORES, NCELL), np.int64)
    percore = []
    for c in range(NCORES):
        m = core == c
        cc = cell[m]
        order = np.argsort(cc, kind="stable")
        percore.append((cc[order], src[m][order], dloc[m][order]))
        counts[c] = np.bincount(cc, minlength=NCELL)
    NE = counts.max(axis=0)

    cell_lo = np.zeros(NCELL, np.int64)
    runs = []
    kmap = []
    groups = {}
    seg_list = []
    nseg = 0
    pos = 0
    for g_i in range(NGB):
        tls = range(g_i * GB, min((g_i + 1) * GB, NTILE))
        for rho_i in range(NRHO):
            run_start = pos
            for t in tls:
                # Cells sorted big->small so 32-aligned placement (PE matmul
                # tile-position rules: base 0 any size, base 64 <=64,
                # base 32/96 <=32) wastes almost nothing.
                for r in range(R):
                    ci = ((g_i * NRHO + rho_i) * NTILE + t) * R + r
                    ne = int(NE[ci])
                    cell_lo[ci] = pos
                    if not ne:
                        continue
                    lo, hi = pos, pos + ne
                    for k in range(lo // 128, (hi - 1) // 128 + 1):
                        s = max(lo, k * 128)
                        e = min(hi, (k + 1) * 128)
                        # full-128 matmul on chunk k with a per-seg MASKED
                        # one-hot column (partition-sliced matmuls crash the
                        # PE on HW when mixed in one accumulation group)
                        seg_list.append((k, nseg, s, e))
                        groups.setdefault((t, r), []).append((k, nseg))
                        nseg += 1
                    pos = hi
            run_slots = pos - run_start
            if run_slots == 0:
                continue
            pos += (-run_slots) % 128
            nk = (pos - run_start) // 128
            # The SWDGE descriptor ring holds 1024 descriptors (hardware
            # fixed); cap each dma_gather at 8 chunks = 1024 rows.
            k0 = run_start // 128
            off = 0
            while off < nk:
                take = min(8, nk - off)
                ri = len(runs)
                runs.append({"gb": g_i, "rho": rho_i, "k0": k0 + off,
                             "nk": take, "slot0": (k0 + off) * 128})
                kmap.extend((ri, b) for b in range(take))
                off += take
    tot_slots = pos
    nchunk = tot_slots // 128

    # per-(sub)run seg-column ranges (contiguous; each seg sits in one chunk)
    for r_ in runs:
        r_["c0col"], r_["c1col"] = nseg, 0
    for (k, col, s, e) in seg_list:
        r_ = runs[kmap[k][0]]
        r_["c0col"] = min(r_["c0col"], col)
        r_["c1col"] = max(r_["c1col"], col + 1)

    run_c0 = np.zeros(len(runs) + 1, np.int64)
    np.cumsum([r_["nk"] * 8 for r_ in runs], out=run_c0[1:])
    idxcols = int(run_c0[-1])

    idx_mats, dl_mats = [], []
    for c in range(NCORES):
        cc, srcs, dlocs = percore[c]
        first_occ = np.searchsorted(cc, np.arange(NCELL))
        pos_e = cell_lo[cc] + (np.arange(len(cc)) - first_occ[cc])
        idx_flat = np.zeros(tot_slots, np.int64)
        idx_flat[pos_e] = srcs % RBASE
        dl_flat = np.full(tot_slots, -1.0, np.float32)
        dl_flat[pos_e] = dlocs
        wr = np.empty((128, max(idxcols, 1)), np.int16)
        for ri, r_ in enumerate(runs):
            fl = idx_flat[r_["slot0"]:r_["slot0"] + r_["nk"] * 128]
            w = fl.reshape(-1, 16).T.astype(np.int16)
            wr[:, run_c0[ri]:run_c0[ri + 1]] = np.tile(w, (8, 1))
        idx_mats.append(wr)
        dl_mat = np.full((128, max(nseg, 1)), -1.0, np.float32)
        for (k, col, s, e) in seg_list:
            colv = np.full(128, -1.0, np.float32)
            colv[s - k * 128:e - k * 128] = dl_flat[s:e]
            dl_mat[:, col] = colv
        dl_mats.append(dl_mat)

    return {"nchunk": nchunk, "nseg": max(nseg, 1), "groups": groups,
            "runs": runs, "kmap": kmap, "run_c0": run_c0,
            "idxcols": max(idxcols, 1), "ngb": NGB,
            "rcmax": max((r_["nk"] for r_ in runs), default=1),
            "ohmax": max((r_["c1col"] - r_["c0col"] for r_ in runs),
                         default=1),
            "idx": idx_mats, "dl": dl_mats}


# ---------------------------------------------------------------------------
# Device program
# ---------------------------------------------------------------------------

def build_program(plans, stages=("prep", "ag01", "l1", "ag234", "l2"),
                  loop_r=None, scopes=False):
    from contextlib import nullcontext
    stages = set(stages)
    noag = "noag" in stages
    nc = bacc.Bacc("TRN2", target_bir_lowering=False, debug=False,
                   num_devices=NCORES, dynamic_dma_scratch_size=16384)

    # --- external inputs (per core) ---
    ext = {}

    def din(name, shape, dt):
        ext[name] = nc.dram_tensor(name, list(shape), dt, kind="ExternalInput")
        return ext[name]

    emb_sl = din("emb_sl", [NLOC, D], TDT)
    gcn_W1 = din("gcn_W1", [D, D], TDT)
    gcn_W2 = din("gcn_W2", [D, D], TDT)
    gcn_b1 = din("gcn_b1", [D, 1], F32)
    gcn_b2r = din("gcn_b2r", [D, D], F32)          # row-broadcast bias
    rg_W1 = din("rg_W1", [D, R_RG * D], TDT)
    rg_W2 = din("rg_W2", [D, R_RG * D], TDT)
    rg_loop1 = din("rg_loop1", [D, D], TDT)
    rg_loop2 = din("rg_loop2", [D, D], TDT)
    rg_b1 = din("rg_b1", [D, 1], F32)
    rg_b2 = din("rg_b2", [D, 1], F32)
    het_W1 = din("het_W1", [D, R_HET * D], TDT)
    het_W2 = din("het_W2", [D, R_HET * D], TDT)    # pre-scaled by 0.25 on host
    het_b1 = din("het_b1", [D, R_HET], F32)
    het_b2 = din("het_b2", [D, R_HET], F32)
    iota_in = din("iota", [D, D], TDT)
    ident_b = din("ident_b", [D, D], TDT)
    ident_f = din("ident_f", [D, D], F32)
    for lname in ("gcn1", "gcn2", "rg1", "rg2", "het1", "het2"):
        p = plans[lname]
        din(f"idx_{lname}", [128, p["idxcols"]], I16)
        din(f"dl_{lname}", [128, p["nseg"]], TDT)

    hcf_out = nc.dram_tensor("hcf", [NLOC, D], F32, kind="ExternalOutput")
    hc_out = nc.dram_tensor("hc", [NLOC, D], F32, kind="ExternalOutput")
    hs_out = nc.dram_tensor("hs", [NLOC, D], F32, kind="ExternalOutput")

    if noag:
        ext_tabs = {}
        for nm in ("emb_full_in", "t1_full_in", "t2_full_in", "h1_full_in",
                   "hs1_full_in"):
            ext_tabs[nm] = nc.dram_tensor(nm, [N, D], TDT, kind="ExternalInput")

    Tanh = mybir.ActivationFunctionType.Tanh
    AG = "AllGather"
    RGROUPS = [list(range(NCORES))]

    with tile.TileContext(nc) as tc:
        with tc.tile_pool(name="consts", bufs=1) as cp, \
             tc.tile_pool(name="gat", bufs=8) as gp, \
             tc.tile_pool(name="idx", bufs=3) as idxp, \
             tc.tile_pool(name="oh", bufs=8) as ohp, \
             tc.tile_pool(name="work", bufs=6) as wp, \
             tc.tile_pool(name="psb", bufs=3, space="PSUM") as psb, \
             tc.tile_pool(name="pss", bufs=3, space="PSUM") as pss, \
             tc.tile_pool(name="ptr", bufs=2, space="PSUM") as ptr, \
             tc.tile_pool(name="dram", bufs=1, space="DRAM") as dp:

            # --- constant tiles ---
            def load_const(name, shape, dt):
                t = cp.tile(list(shape), dt, tag=name)
                nc.sync.dma_start(out=t[:], in_=ext[name][:])
                return t

            iota_t = load_const("iota", [D, D], TDT)
            identb_t = load_const("ident_b", [D, D], TDT)
            identf_t = load_const("ident_f", [D, D], F32)
            gW1_t = load_const("gcn_W1", [D, D], TDT)
            gW2_t = load_const("gcn_W2", [D, D], TDT)
            gb1_t = load_const("gcn_b1", [D, 1], F32)
            gb2r_t = load_const("gcn_b2r", [D, D], F32)
            rW1_t = load_const("rg_W1", [D, R_RG * D], TDT)
            rW2_t = load_const("rg_W2", [D, R_RG * D], TDT)
            rL1_t = load_const("rg_loop1", [D, D], TDT)
            rL2_t = load_const("rg_loop2", [D, D], TDT)
            rb1_t = load_const("rg_b1", [D, 1], F32)
            rb2_t = load_const("rg_b2", [D, 1], F32)
            hW1_t = load_const("het_W1", [D, R_HET * D], TDT)
            hW2_t = load_const("het_W2", [D, R_HET * D], TDT)
            hb1_t = load_const("het_b1", [D, R_HET], F32)
            hb2_t = load_const("het_b2", [D, R_HET], F32)
            meta = {}
            for lname in ("gcn1", "gcn2", "rg1", "rg2", "het1", "het2"):
                p = plans[lname]
                meta[lname] = (
                    None,  # idx stays in DRAM; streamed per gather-batch
                    load_const(f"dl_{lname}", [128, p["nseg"]], TDT),
                )

            # --- internal DRAM ---
            emb_bounce = dp.tile([NLOC, D], TDT, tag="emb_b")
            emb_full = dp.tile([N, D], TDT, tag="emb_f", addr_space="Shared")
            loop_ctx = (tc.For_i(0, loop_r, 1)
                        if (loop_r and "agrep" not in stages) else None)
            if loop_ctx:
                loop_ctx.__enter__()
            t1_bounce = dp.tile([NLOC, D], TDT, tag="t1_b")
            t1_full = dp.tile([N, D], TDT, tag="t1_f", addr_space="Shared")
            t2_bounce = dp.tile([NLOC, D], TDT, tag="t2_b")
            t2_full = dp.tile([N, D], TDT, tag="t2_f", addr_space="Shared")
            h1_bounce = dp.tile([NLOC, D], TDT, tag="h1_b")
            h1_full = dp.tile([N, D], TDT, tag="h1_f", addr_space="Shared")
            hs1_bounce = dp.tile([NLOC, D], TDT, tag="hs1_b")
            hs1_full = dp.tile([N, D], TDT, tag="hs1_f", addr_space="Shared")
            embT_dram = dp.tile([D, NLOC], TDT, tag="embT")
            h1T_dram = dp.tile([D, NLOC], TDT, tag="h1T")

            def _scope(name):
                return nc.named_scope(name, notify=True) if scopes \
                    else nullcontext()

            def _ag(src, dst):
                nc.gpsimd.collective_compute(
                    AG, mybir.AluOpType.bypass, replica_groups=RGROUPS,
                    ins=[src.opt()], outs=[dst.opt()])

            # --- prep: embT tiles, T1 = emb @ W1, bounces ---
            prep_scope = _scope("prep")
            prep_scope.__enter__()
            nc.sync.dma_start(out=emb_bounce[:], in_=emb_sl[:])
            # emb AG fires as soon as the bounce copy lands, overlapping the
            # prep matmul loop; rg1/het1 gathers can then start during prep.
            if "ag01" in stages:
                _ag(emb_bounce, emb_full)
            for t in range(NTILE):
                w = _tw(t)
                e_sb = wp.tile([128, D], TDT, tag="embt")
                if w < 128:
                    nc.vector.memset(e_sb[:], 0.0)
                nc.sync.dma_start(out=e_sb[:w, :], in_=emb_sl[t * 128:t * 128 + w, :])
                trp = ptr.tile([128, D], TDT, tag="ptr")
                nc.tensor.transpose(out=trp[:], in_=e_sb[:], identity=identb_t[:])
                eT = wp.tile([128, D], TDT, tag="eT")
                nc.vector.tensor_copy(out=eT[:], in_=trp[:])
                nc.sync.dma_start(out=embT_dram[:, t * 128:t * 128 + w],
                                  in_=eT[:, :w])
                t1p = pss.tile([128, D], F32, tag="pss")
                nc.tensor.matmul(out=t1p[:], lhsT=eT[:], rhs=gW1_t[:],
                                 start=True, stop=True)
                t1sb = wp.tile([128, D], TDT, tag="t1sb")
                nc.vector.tensor_copy(out=t1sb[:], in_=t1p[:])
                nc.sync.dma_start(out=t1_bounce[t * 128:t * 128 + w, :],
                                  in_=t1sb[:w, :])

            prep_scope.__exit__(None, None, None)

            if "ag01" in stages:
                with _scope("ag01"):
                    _ag(t1_bounce, t1_full)

            # --- shared layer machinery ---
            # Gathers: one dma_gather (InstDMAGatherAnt) per (gather-batch,
            # src-range) run — batches hundreds of row-descriptors into one
            # Pool-engine desc-gen pass (994ns fixed + 0.34ns/row) instead of
            # one indirect DMA per 128 rows.
            RCMAX = max(p["rcmax"] for p in plans.values())

            def _gb_spans(p):
                spans = {}
                for ri, r_ in enumerate(p["runs"]):
                    s = spans.setdefault(r_["gb"], [ri, ri + 1])
                    s[1] = ri + 1
                return spans

            IDXW = 1
            for p in plans.values():
                for (r0, r1) in _gb_spans(p).values():
                    IDXW = max(IDXW, int(p["run_c0"][r1] - p["run_c0"][r0]))

            def gather_layer(lname, table):
                plan = plans[lname]
                runs, kmap, run_c0 = plan["runs"], plan["kmap"], plan["run_c0"]
                spans = _gb_spans(plan)
                idx_dram = ext[f"idx_{lname}"]
                tiles = {}

                def prepare(g_i):
                    if g_i not in spans:
                        return
                    r0, r1 = spans[g_i]
                    c_lo, c_hi = int(run_c0[r0]), int(run_c0[r1])
                    it = idxp.tile([128, IDXW], I16, tag="idxw")
                    nc.sync.dma_start(out=it[:, :c_hi - c_lo],
                                      in_=idx_dram[:, c_lo:c_hi])
                    for ri in range(r0, r1):
                        r_ = runs[ri]
                        gt = gp.tile([128, RCMAX * D], TDT, tag="gat")
                        nidx = r_["nk"] * 128
                        base = r_["rho"] * RBASE
                        nc.gpsimd.dma_gather(
                            out_ap=gt[:, :r_["nk"] * D].rearrange(
                                "p (b d) -> p b d", b=r_["nk"]),
                            in_ap=table[base:base + RBASE, :],
                            idxs_ap=it[:, int(run_c0[ri]) - c_lo:
                                       int(run_c0[ri + 1]) - c_lo],
                            num_idxs=nidx, num_idxs_reg=nidx, elem_size=D)
                        tiles[ri] = gt

                def gather(k):
                    ri, b = kmap[k]
                    return tiles[ri][:, b * D:(b + 1) * D]
                return prepare, gather

            # One-hot builds: masked per-seg columns, built wide per
            # (gb, range) run in one is_equal; alternate DVE / Pool to
            # balance engine load (Pool takes 2 of 5).
            OHMAX = max(p["ohmax"] for p in plans.values())
            _oh_flip = [0]

            def oh_layer(lname):
                plan = plans[lname]
                runs = plan["runs"]
                spans = _gb_spans(plan)
                _, dl_t = meta[lname]
                blocks = {}          # run index -> (oh tile, c0col)

                def prepare(g_i):
                    if g_i not in spans:
                        return
                    r0, r1 = spans[g_i]
                    for ri in range(r0, r1):
                        r_ = runs[ri]
                        c0, c1 = r_["c0col"], r_["c1col"]
                        S = c1 - c0
                        if S <= 0:
                            continue
                        oh = ohp.tile([128, OHMAX * D], TDT, tag="ohw")
                        eng = nc.vector
                        _oh_flip[0] += 1
                        eng.tensor_tensor(
                            out=oh[:, :S * D].rearrange("p (s d) -> p s d",
                                                        s=S),
                            in0=dl_t[:, c0:c1].unsqueeze(2).broadcast_to(
                                [128, S, D]),
                            in1=iota_t[:, :].unsqueeze(1).broadcast_to(
                                [128, S, D]),
                            op=mybir.AluOpType.is_equal)
                        blocks[ri] = (oh, c0)

                def resolve(k, col, kmap=plan["kmap"]):
                    oh, c0 = blocks[kmap[k][0]]
                    return oh[:, (col - c0) * D:(col - c0 + 1) * D]
                return prepare, resolve

            def accum_group(segs, gather, ohb, bank, bcol, transposed):
                """Accumulate one (tile, rel) group into bank[:, bcol:bcol+128].

                transposed=True -> out[f, d] (lhsT=msgs, rhs=onehot)
                transposed=False -> out[d, f] (lhsT=onehot, rhs=msgs)
                """
                for si, (k, col) in enumerate(segs):
                    gt = gather(k)
                    oh = ohb(k, col)
                    lhsT, rhs = (gt, oh) if transposed else (oh, gt)
                    nc.tensor.matmul(out=bank[:, bcol:bcol + 128],
                                     lhsT=lhsT, rhs=rhs,
                                     start=(si == 0), stop=(si == len(segs) - 1))

            # =========== GCN layer 1 (gathers T1; aggT supertiles) ==========
            def emit_gcn1():
                lname = "gcn1"
                plan = plans[lname]
                prepare, gather = gather_layer(lname, t1_full)
                ohprep, ohres = oh_layer(lname)
                for st in range(plan["ngb"]):
                    tls = list(range(st * GB, min(st * GB + GB, NTILE)))
                    prepare(st)
                    ohprep(st)
                    bank = psb.tile([128, 512], F32, tag="psb")
                    for j, t in enumerate(tls):
                        segs = plan["groups"].get((t, 0))
                        if not segs:
                            nc.vector.memset(bank[:, j * 128:(j + 1) * 128], 0.0)
                            continue
                        accum_group(segs, gather, ohres, bank, j * 128, True)
                    w = 128 * len(tls)
                    h1T = wp.tile([128, 512], TDT, tag="h1Tst")
                    nc.scalar.activation(h1T[:, :w], bank[:, :w], Tanh,
                                         bias=gb1_t[:], scale=1.0)
                    for j, t in enumerate(tls):
                        tp = pss.tile([128, D], F32, tag="pss")
                        nc.tensor.matmul(out=tp[:],
                                         lhsT=h1T[:, j * 128:(j + 1) * 128],
                                         rhs=gW2_t[:], start=True, stop=True)
                        tsb = wp.tile([128, D], TDT, tag="t2sb")
                        nc.vector.tensor_copy(out=tsb[:], in_=tp[:])
                        nc.sync.dma_start(
                            out=t2_bounce[t * 128:t * 128 + _tw(t), :],
                            in_=tsb[:_tw(t), :])

            # =========== GCN layer 2 (gathers T2; agg per tile) =============
            def emit_gcn2():
                lname = "gcn2"
                plan = plans[lname]
                prepare, gather = gather_layer(lname, t2_full)
                ohprep, ohres = oh_layer(lname)
                for t in range(NTILE):
                    if t % GB == 0:
                        prepare(t // GB)
                        ohprep(t // GB)
                    segs = plan["groups"].get((t, 0))
                    pt = pss.tile([128, D], F32, tag="pss")
                    if not segs:
                        nc.vector.memset(pt[:], 0.0)
                    else:
                        accum_group(segs, gather, ohres, pt, 0, False)
                    tmp = wp.tile([128, D], F32, tag="g2tmp")
                    nc.vector.tensor_add(out=tmp[:], in0=pt[:], in1=gb2r_t[:])
                    ot = wp.tile([128, D], F32, tag="g2out")
                    nc.scalar.activation(ot[:], tmp[:], Tanh)
                    nc.sync.dma_start(out=hcf_out[t * 128:t * 128 + _tw(t), :],
                                      in_=ot[:_tw(t), :])

            # =========== RGCN layer (B banks per rel + transforms) ==========
            def emit_rg(lname, table, xT_src, W_t, loop_t, b_t, first):
                plan = plans[lname]
                prepare, gather = gather_layer(lname, table)
                ohprep, ohres = oh_layer(lname)
                for t in range(NTILE):
                    w = _tw(t)
                    if t % GB == 0:
                        prepare(t // GB)
                        ohprep(t // GB)
                    quads = []
                    for qi in range(2):
                        q = psb.tile([128, 512], F32, tag="psb")
                        quads.append(q)
                    for r in range(R_RG):
                        segs = plan["groups"].get((t, r))
                        q, qc = quads[r // 4], (r % 4) * 128
                        if not segs:
                            nc.vector.memset(q[:, qc:qc + 128], 0.0)
                        else:
                            accum_group(segs, gather, ohres, q, qc, True)
                    stages = []
                    for qi in range(2):
                        s = wp.tile([128, 512], TDT, tag="stage")
                        nc.scalar.copy(out=s[:], in_=quads[qi][:])
                        stages.append(s)
                    xT_t = wp.tile([128, D], TDT, tag="xTt")
                    nc.sync.dma_start(out=xT_t[:, :w],
                                      in_=xT_src[:, t * 128:t * 128 + w])
                    ot = pss.tile([128, D], F32, tag="pss")
                    nc.tensor.matmul(out=ot[:], lhsT=loop_t[:], rhs=xT_t[:],
                                     start=True, stop=False)
                    for r in range(R_RG):
                        nc.tensor.matmul(
                            out=ot[:], lhsT=W_t[:, r * 128:(r + 1) * 128],
                            rhs=stages[r // 4][:, (r % 4) * 128:(r % 4 + 1) * 128],
                            start=False, stop=(r == R_RG - 1))
                    if first:
                        hT = wp.tile([128, D], TDT, tag="hTb")
                        nc.scalar.activation(hT[:], ot[:], Tanh, bias=b_t[:],
                                             scale=1.0)
                        nc.sync.dma_start(
                            out=h1T_dram[:, t * 128:t * 128 + w], in_=hT[:, :w])
                        trp = ptr.tile([128, D], TDT, tag="ptr")
                        nc.tensor.transpose(out=trp[:], in_=hT[:],
                                            identity=identb_t[:])
                        hsb = wp.tile([128, D], TDT, tag="hsbb")
                        nc.vector.tensor_copy(out=hsb[:], in_=trp[:])
                        nc.sync.dma_start(out=h1_bounce[t * 128:t * 128 + w, :],
                                          in_=hsb[:w, :])
                    else:
                        hTf = wp.tile([128, D], F32, tag="hTf")
                        nc.scalar.activation(hTf[:], ot[:], Tanh, bias=b_t[:],
                                             scale=1.0)
                        trp = ptr.tile([128, D], F32, tag="ptr")
                        nc.tensor.transpose(out=trp[:], in_=hTf[:],
                                            identity=identf_t[:])
                        hsb = wp.tile([128, D], F32, tag="hsbf")
                        nc.vector.tensor_copy(out=hsb[:], in_=trp[:])
                        nc.sync.dma_start(out=hc_out[t * 128:t * 128 + w, :],
                                          in_=hsb[:w, :])

            # =========== Hetero layer (4 rels, mean of tanh) ================
            def emit_het(lname, table, W_t, b_t, first):
                plan = plans[lname]
                prepare, gather = gather_layer(lname, table)
                ohprep, ohres = oh_layer(lname)
                for t in range(NTILE):
                    w = _tw(t)
                    if t % GB == 0:
                        prepare(t // GB)
                        ohprep(t // GB)
                    quad = psb.tile([128, 512], F32, tag="psb")
                    for r in range(R_HET):
                        segs = plan["groups"].get((t, r))
                        if not segs:
                            nc.vector.memset(quad[:, r * 128:(r + 1) * 128], 0.0)
                        else:
                            accum_group(segs, gather, ohres, quad, r * 128, True)
                    stage = wp.tile([128, 512], TDT, tag="stage")
                    nc.scalar.copy(out=stage[:], in_=quad[:])
                    acc = wp.tile([128, D], F32, tag="hacc")
                    for r in range(R_HET):
                        otr = pss.tile([128, D], F32, tag="pss")
                        nc.tensor.matmul(
                            out=otr[:], lhsT=W_t[:, r * 128:(r + 1) * 128],
                            rhs=stage[:, r * 128:(r + 1) * 128],
                            start=True, stop=True)
                        if r == 0:
                            nc.scalar.activation(acc[:], otr[:], Tanh,
                                                 bias=b_t[:, 0:1], scale=1.0)
                        else:
                            tmp = wp.tile([128, D], F32, tag="htmp")
                            nc.scalar.activation(tmp[:], otr[:], Tanh,
                                                 bias=b_t[:, r:r + 1], scale=1.0)
                            nc.vector.tensor_add(out=acc[:], in0=acc[:],
                                                 in1=tmp[:])
                    if first:
                        # no 0.25 scale: folded into het_W2 on host
                        hsT = wp.tile([128, D], TDT, tag="hTb")
                        nc.vector.tensor_copy(out=hsT[:], in_=acc[:])
                        trp = ptr.tile([128, D], TDT, tag="ptr")
                        nc.tensor.transpose(out=trp[:], in_=hsT[:],
                                            identity=identb_t[:])
                        hsb = wp.tile([128, D], TDT, tag="hsbb")
                        nc.vector.tensor_copy(out=hsb[:], in_=trp[:])
                        nc.sync.dma_start(out=hs1_bounce[t * 128:t * 128 + w, :],
                                          in_=hsb[:w, :])
                    else:
                        hsT = wp.tile([128, D], F32, tag="hTf")
                        nc.vector.tensor_scalar_mul(hsT[:], acc[:], 0.25)
                        trp = ptr.tile([128, D], F32, tag="ptr")
                        nc.tensor.transpose(out=trp[:], in_=hsT[:],
                                            identity=identf_t[:])
                        hsb = wp.tile([128, D], F32, tag="hsbf")
                        nc.vector.tensor_copy(out=hsb[:], in_=trp[:])
                        nc.sync.dma_start(out=hs_out[t * 128:t * 128 + w, :],
                                          in_=hsb[:w, :])

            # --- emit layers ---
            if noag:
                emb_full = ext_tabs["emb_full_in"]
                t1_full = ext_tabs["t1_full_in"]
                t2_full = ext_tabs["t2_full_in"]
                h1_full = ext_tabs["h1_full_in"]
                hs1_full = ext_tabs["hs1_full_in"]
            # Each layer-1 AG fires right after its producer layer so the
            # transfer overlaps the remaining layer-1 compute.
            if "l1" in stages or "l1rg" in stages:
                with _scope("l1_rg"):
                    emit_rg("rg1", emb_full, embT_dram, rW1_t, rL1_t, rb1_t,
                            True)
                if "ag234" in stages:
                    _ag(h1_bounce, h1_full)
            if "l1" in stages or "l1het" in stages:
                with _scope("l1_het"):
                    emit_het("het1", emb_full, hW1_t, hb1_t, True)
                if "ag234" in stages:
                    _ag(hs1_bounce, hs1_full)
            if "l1" in stages or "l1gcn" in stages:
                with _scope("l1_gcn"):
                    emit_gcn1()
                if "ag234" in stages:
                    _ag(t2_bounce, t2_full)

            if "l2" in stages or "l2rg" in stages:
                with _scope("l2_rg"):
                    emit_rg("rg2", h1_full, h1T_dram, rW2_t, rL2_t, rb2_t,
                            False)
            if "l2" in stages or "l2het" in stages:
                with _scope("l2_het"):
                    emit_het("het2", hs1_full, hW2_t, hb2_t, False)
            if "l2" in stages or "l2gcn" in stages:
                with _scope("l2_gcn"):
                    emit_gcn2()
            if loop_ctx:
                loop_ctx.__exit__(None, None, None)

    nc.compile()
    return nc


# ---------------------------------------------------------------------------
# Runner (PJRT via axon)
# ---------------------------------------------------------------------------

class _Runner:
    """One execute + one await per run.

    The axon tunnel costs ~70ms per client-side await RPC (independent of
    data size or device work), so the run path is: a single bass_exec
    dispatch over all 8 cores, then a single jax.block_until_ready. The
    kernel writes every element of each ExternalOutput, so no pre-zeroed
    output operands are passed (PJRT allocates the result buffers and the
    NEFF fills them).
    """

    def __init__(self, nc, n_cores):
        install_neuronx_cc_hook()
        self.n_cores = n_cores
        partition_name = (nc.partition_id_tensor.name
                          if nc.partition_id_tensor else None)
        in_names, out_names, out_avals = [], [], []
        for alloc in nc.m.functions[0].allocations:
            if not isinstance(alloc, mybir.MemoryLocationSet):
                continue
            name = alloc.memorylocations[0].name
            if alloc.kind == "ExternalInput":
                if name != partition_name:
                    in_names.append(name)
            elif alloc.kind == "ExternalOutput":
                shape = tuple(alloc.tensor_shape)
                dtype = mybir.dt.np(alloc.dtype)
                out_avals.append(jax.core.ShapedArray(shape, dtype))
                out_names.append(name)
        self.in_names, self.out_names = in_names, out_names
        self.out_avals = out_avals
        n_params, n_outs = len(in_names), len(out_avals)
        all_in = list(in_names)
        if partition_name is not None:
            all_in.append(partition_name)

        def _body(*args):
            operands = list(args)
            if partition_name is not None:
                operands.append(partition_id_tensor())
            return tuple(_bass_exec_p.bind(
                *operands, out_avals=tuple(out_avals), in_names=tuple(all_in),
                out_names=tuple(out_names), lowering_input_output_aliases=(),
                sim_require_finite=True, sim_require_nnan=True, nc=nc))

        devices = jax.devices()[:n_cores]
        self.mesh = Mesh(np.asarray(devices), ("core",))
        in_specs = (PartitionSpec("core"),) * n_params
        out_specs = (PartitionSpec("core"),) * n_outs
        self._body = _body
        self._specs = (in_specs, out_specs)
        self.fn = None
        self.sharding = NamedSharding(self.mesh, PartitionSpec("core"))

    def _ensure_compiled(self):
        # AOT-compile with the bass effect suppressed so repeat dispatches go
        # through JAX's C++ fast path (the effectful path adds per-call
        # Python token machinery).
        if self.fn is None:
            in_specs, out_specs = self._specs

            def _compile():
                return jax.jit(
                    shard_map(self._body, mesh=self.mesh, in_specs=in_specs,
                              out_specs=out_specs, check_rep=False),
                    keep_unused=True).lower(*self.dev_in).compile()

            self.fn = fast_dispatch_compile(_compile)

    def put_inputs(self, in_maps):
        n = self.n_cores
        per_core = [[np.asarray(m[k]) for k in self.in_names] for m in in_maps]
        self.dev_in = [
            jax.device_put(
                np.concatenate([per_core[c][i] for c in range(n)], axis=0),
                self.sharding)
            for i in range(len(self.in_names))
        ]
        jax.block_until_ready(self.dev_in)

    def run(self, fetch=True):
        n = self.n_cores
        self._ensure_compiled()
        outs = self.fn(*self.dev_in)
        jax.block_until_ready(outs)
        if not fetch:
            return None
        return [
            {name: np.asarray(outs[i]).reshape(n, *self.out_avals[i].shape)[c]
             for i, name in enumerate(self.out_names)}
            for c in range(n)
        ]


# ---------------------------------------------------------------------------
# Entry point
# ---------------------------------------------------------------------------

_LAST_RUNNER = None


def build_all(inputs, stages=("prep", "ag01", "l1", "ag234", "l2"),
              loop_r=None, scopes=False):
    """Pack edges + build program + per-core input maps. Returns (nc, in_maps)."""
    gcn_src1 = inputs["gcn_src1"]; gcn_dst1 = inputs["gcn_dst1"]
    gcn_src2 = inputs["gcn_src2"]; gcn_dst2 = inputs["gcn_dst2"]
    rg_src1 = inputs["rg_src1"]; rg_dst1 = inputs["rg_dst1"]
    rg_et1 = inputs["rg_et1"]
    rg_src2 = inputs["rg_src2"]; rg_dst2 = inputs["rg_dst2"]
    rg_et2 = inputs["rg_et2"]
    het_src1 = inputs["het_src1"]; het_dst1 = inputs["het_dst1"]
    het_src2 = inputs["het_src2"]; het_dst2 = inputs["het_dst2"]
    emb = inputs["emb"]
    gcn_W1 = inputs["gcn_W1"]; gcn_b1 = inputs["gcn_b1"]
    gcn_W2 = inputs["gcn_W2"]; gcn_b2 = inputs["gcn_b2"]
    rg_W1 = inputs["rg_W1"]; rg_loop1 = inputs["rg_loop1"]
    rg_b1 = inputs["rg_b1"]
    rg_W2 = inputs["rg_W2"]; rg_loop2 = inputs["rg_loop2"]
    rg_b2 = inputs["rg_b2"]
    het_W1 = inputs["het_W1"]; het_b1 = inputs["het_b1"]
    het_W2 = inputs["het_W2"]; het_b2 = inputs["het_b2"]
    emb = np.asarray(emb, np.float32)

    # hetero edge lists: concatenate the 4 relations with rel tags
    def het_edges(srcs, dsts):
        s = np.concatenate([np.asarray(srcs[r]).ravel() for r in range(R_HET)])
        d = np.concatenate([np.asarray(dsts[r]).ravel() for r in range(R_HET)])
        r = np.concatenate([np.full(np.asarray(srcs[r]).size, r, np.int64)
                            for r in range(R_HET)])
        return s, d, r

    hs1_, hd1_, hr1_ = het_edges(het_src1, het_dst1)
    hs2_, hd2_, hr2_ = het_edges(het_src2, het_dst2)

    plans = {
        "gcn1": pack_layer(gcn_src1, gcn_dst1, None, 1),
        "gcn2": pack_layer(gcn_src2, gcn_dst2, None, 1),
        "rg1": pack_layer(rg_src1, rg_dst1, rg_et1, R_RG),
        "rg2": pack_layer(rg_src2, rg_dst2, rg_et2, R_RG),
        "het1": pack_layer(hs1_, hd1_, hr1_, R_HET),
        "het2": pack_layer(hs2_, hd2_, hr2_, R_HET),
    }

    nc = build_program(plans, stages=stages, loop_r=loop_r, scopes=scopes)

    iota_np = np.broadcast_to(np.arange(D, dtype=np.float32), (D, D))
    shared = {
        "gcn_W1": np.asarray(gcn_W1).astype(TNP),
        "gcn_W2": np.asarray(gcn_W2).astype(TNP),
        "gcn_b1": np.asarray(gcn_b1, np.float32).reshape(D, 1),
        "gcn_b2r": np.broadcast_to(np.asarray(gcn_b2, np.float32), (D, D)).copy(),
        "rg_W1": np.concatenate([np.asarray(rg_W1)[r] for r in range(R_RG)],
                                axis=1).astype(TNP),
        "rg_W2": np.concatenate([np.asarray(rg_W2)[r] for r in range(R_RG)],
                                axis=1).astype(TNP),
        "rg_loop1": np.asarray(rg_loop1).astype(TNP),
        "rg_loop2": np.asarray(rg_loop2).astype(TNP),
        "rg_b1": np.asarray(rg_b1, np.float32).reshape(D, 1),
        "rg_b2": np.asarray(rg_b2, np.float32).reshape(D, 1),
        "het_W1": np.concatenate([np.asarray(het_W1)[r] for r in range(R_HET)],
                                 axis=1).astype(TNP),
        "het_W2": np.concatenate([0.25 * np.asarray(het_W2)[r]
                                  for r in range(R_HET)], axis=1).astype(TNP),
        "het_b1": np.ascontiguousarray(np.asarray(het_b1, np.float32).T),
        "het_b2": np.ascontiguousarray(np.asarray(het_b2, np.float32).T),
        "iota": iota_np.astype(TNP),
        "ident_b": np.eye(D, dtype=TNP),
        "ident_f": np.eye(D, dtype=np.float32),
    }

    in_maps = []
    for c in range(NCORES):
        m = dict(shared)
        m["emb_sl"] = emb[c * NLOC:(c + 1) * NLOC, :].astype(TNP)
        for lname in ("gcn1", "gcn2", "rg1", "rg2", "het1", "het2"):
            m[f"idx_{lname}"] = plans[lname]["idx"][c]
            m[f"dl_{lname}"] = plans[lname]["dl"][c].astype(TNP)
        in_maps.append(m)
    return nc, in_maps


def kernel(gcn_src1, gcn_dst1, gcn_src2, gcn_dst2,
           rg_src1, rg_dst1, rg_et1, rg_src2, rg_dst2, rg_et2,
           het_src1, het_dst1, het_src2, het_dst2,
           emb, gcn_W1, gcn_b1, gcn_W2, gcn_b2,
           rg_W1, rg_loop1, rg_b1, rg_W2, rg_loop2, rg_b2,
           het_W1, het_b1, het_W2, het_b2):
    nc, in_maps = build_all(dict(
        gcn_src1=gcn_src1, gcn_dst1=gcn_dst1, gcn_src2=gcn_src2,
        gcn_dst2=gcn_dst2, rg_src1=rg_src1, rg_dst1=rg_dst1, rg_et1=rg_et1,
        rg_src2=rg_src2, rg_dst2=rg_dst2, rg_et2=rg_et2, het_src1=het_src1,
        het_dst1=het_dst1, het_src2=het_src2, het_dst2=het_dst2, emb=emb,
        gcn_W1=gcn_W1, gcn_b1=gcn_b1, gcn_W2=gcn_W2, gcn_b2=gcn_b2,
        rg_W1=rg_W1, rg_loop1=rg_loop1, rg_b1=rg_b1, rg_W2=rg_W2,
        rg_loop2=rg_loop2, rg_b2=rg_b2, het_W1=het_W1, het_b1=het_b1,
        het_W2=het_W2, het_b2=het_b2))
    runner = _Runner(nc, NCORES)
    global _LAST_RUNNER
    _LAST_RUNNER = runner
    runner.put_inputs(in_maps)
    res = runner.run()

    hcf = np.concatenate([res[c]["hcf"] for c in range(NCORES)], axis=0)
    hc = np.concatenate([res[c]["hc"] for c in range(NCORES)], axis=0)
    hs = np.concatenate([res[c]["hs"] for c in range(NCORES)], axis=0)
    return (hcf, hc, hs)

